# revision 13
# baseline (speedup 1.0000x reference)
"""Trainium2 Bass kernel for nn_Conv2d_72052371357971.

Text-CNN style conv stack: three conv groups (k=1,2,3) over [N,3,256]
windows + per-group max-pool, concatenated to [N,256].

Strategy (pure data parallel across 8 NeuronCores):
  * All three conv groups fold into ONE [768, 406] weight matrix over the
    flattened window (3*256 channels), block-sparse by k-subtile support:
      A = y1h0 (j0,j1)   D = y2h0 (j0..j3)   F = o3 (j0..j5)
      B = y1h1 (j2,j3)   E = y2h1 (j2..j5)   C = y1h2 (j4,j5)
  * v10: ONE PSUM bank per batch tile, one accumulation chain.  PSUM
    `has_written` is per-element: the start=True matmul clears the bank;
    later accumulating matmuls OVERWRITE not-yet-written columns (first
    touch) and accumulate written ones.  Column order [A D F B E C] makes
    every k-subtile's active span contiguous with ZERO padding:
      j0/j1: cols   0:256  (A D F)
      j2/j3: cols  50:356  (D F B E)
      j4/j5: cols 100:256 + 306:406  (F, E C; two spans share one
             LDWEIGHTS since the stationary x-subtile is unchanged)
    1636 streamed columns/tile (vs 2436 dense) -- measured ~0.445 ns/col;
    weight swaps and LDWEIGHTS are fully hidden, issue cost ~4.5ns/MM.
  * x is fp8e3 (e3m4) -- halves HBM in-traffic (DMA would otherwise
    co-bottleneck: in+out share the ~350GB/s per-core HBM path).  W stays
    fp16; the PE accepts mixed fp8e3-stationary x fp16-moving operands at
    the same 1 col/cycle.  PSUM accumulates fp32.  Max rel err 1.0e-2
    (fp8e4 DoubleRow would halve PE time but measures 2.4e-2 > 2e-2).
  * Post-process per 128-row tile: ONE ScalarE 256-col copy PSUM->out
    ([A D F] -> [o1 o2 o3] slots), then VectorE max with [B E] (100 cols)
    and with [C] (50 cols) in place.  DMA streams out fp16 rows; host
    upcasts to fp32.
"""

import numpy as np

import concourse.bacc as bacc
import concourse.mybir as mybir
import concourse.tile as tile
from concourse.bass import ds
from concourse.bass_utils import run_bass_kernel_spmd

# Problem shapes (hardcoded per contract)
N = 65536
NCORES = 8
B = N // NCORES           # 8192 batch rows per core
TB = 128                  # batch tile (PSUM partition dim)
TPS = 8                   # batch tiles per super-tile
SUP = B // (TPS * TB)     # 8 super-tiles per core
K = 768                   # contraction: 3 positions x 256 channels
KS = K // 128             # 6 K-subtiles
F = 406                   # pre-pool filters: 3*50 + 2*50 + 156
FO = 256                  # output filters after pooling

import ml_dtypes

_F32 = mybir.dt.float32
_F16 = mybir.dt.float16
_F16_NP = np.float16
_BF16 = mybir.dt.bfloat16
_BF16_NP = ml_dtypes.bfloat16
_E3 = mybir.dt.float8e3
_E3_NP = ml_dtypes.float8_e3m4
_cache = {}

# v9 span schedule: (j, wcol0, ncols, start).  Weight-buffer column layout
# [A 0:50 | D 50:100 | F 100:256 | B 256:306 | E 306:356 | C 356:406],
# same order in every k-subtile segment.  Single PSUM bank; first-touch
# columns are overwritten via per-element has_written.
SPANS = {
    "v9": [
        (0, 0, 256, True),
        (1, 0, 256, False),
        (2, 50, 306, False),
        (3, 50, 306, False),
        (4, 100, 306, False),
        (5, 100, 306, False),
    ],
    # v10: j4/j5 split into two spans sharing one LDWEIGHTS (B-region pad
    # eliminated): 1636 streamed cols, 8 matmuls, 6 weight swaps.
    "v10": [
        (0, 0, 256, True),
        (1, 0, 256, False),
        (2, 50, 306, False),
        (3, 50, 306, False),
        (4, 100, 156, False), (4, 306, 100, False),
        (5, 100, 156, False), (5, 306, 100, False),
    ],
}
SPANS["v11"] = SPANS["v10"]
# x dtype per variant: fp8e3 (e3m4) halves HBM in-traffic; the PE matmul
# mixes fp8e3 stationary x with 16-bit moving weights at 1 col/cycle.
XDT = {"v9": (_F16, _F16_NP), "v10": (_E3, _E3_NP), "v11": (_E3, _E3_NP)}
WDT = {"v9": (_F16, _F16_NP), "v10": (_F16, _F16_NP), "v11": (_BF16, _BF16_NP)}


def _build_nc(
    reps=1,
    has_bias=True,
    variant="v9",
    xbufs=2,
    obufs=2,
    pbufs=4,
    tgroup=1,  # batch tiles per PSUM group (1: one bank/tile; 2: two banks,
               # post-ops batched across the pair via 3D strided APs)
    store_eng="scalar",  # engine issuing the output-store DMA
    probe=None,  # timing diagnostics, comma-separated: 'dma' no compute;
                 # 'noin' no x loads; 'noout' no out stores; 'nopost' no
                 # ACT/DVE post-ops; 'samex' all matmuls share one lhsT
):
    probes = set((probe or "").split(","))
    spans = SPANS[variant]
    xdt, _ = XDT[variant]
    wdt, _ = WDT[variant]
    nc = bacc.Bacc("TRN2", target_bir_lowering=False, debug=False)

    x_d = nc.dram_tensor("x", [SUP, 128, TPS * KS * TB], xdt, kind="ExternalInput")
    w_d = nc.dram_tensor("w", [128, KS * F], wdt, kind="ExternalInput")
    # bias row and a ones row for the K=1 bias matmul
    b_d = nc.dram_tensor("b", [1, F + TB], wdt, kind="ExternalInput")
    # p-major layout: the store is contiguous per partition (4KB chunks);
    # host reorders [s,p,t,f] -> batch order for free
    o_d = nc.dram_tensor("o", [SUP, 128, TPS, FO], _F16, kind="ExternalOutput")

    with tile.TileContext(nc) as tc:
        with (
            tc.tile_pool(name="const", bufs=1) as constp,
            tc.tile_pool(name="xp", bufs=xbufs) as xp,
            tc.tile_pool(name="op", bufs=obufs) as op,
            tc.tile_pool(name="ps", bufs=pbufs, space="PSUM") as psp,
        ):
            wt = constp.tile([128, KS * F], wdt)
            nc.sync.dma_start(wt[:], w_d[:])
            if "noin" in probes:
                # single resident x tile: marginal rep traffic excludes loads
                xt0 = constp.tile([128, TPS * KS * TB], xdt)
                nc.sync.dma_start(xt0[:], x_d[0])
            if has_bias:
                bt = constp.tile([1, F + TB], wdt)
                nc.sync.dma_start(bt[:], b_d[:])
                brow = bt[:, ds(0, F)]
                ones = bt[:, ds(F, TB)]

            for s in [si for _ in range(reps) for si in range(SUP)]:
                if "noin" in probes:
                    xt = xt0
                else:
                    xt = xp.tile([128, TPS * KS * TB], xdt)
                    nc.sync.dma_start(xt[:], x_d[s])
                ot = op.tile([128, TPS * FO], _F16)
                if "dma" in probes:
                    # trivial write so the out store has a defined producer
                    nc.vector.tensor_copy(ot[:], xt[:, ds(0, TPS * FO)])
                for tg in range(TPS // tgroup) if "dma" not in probes else []:
                    acc = psp.tile([128, 512 * tgroup], _F32, tag="ps", name="acc")
                    for ti in range(tgroup):
                        t = tg * tgroup + ti
                        p0 = ti * 512
                        tspans = spans
                        if "split" in probes:  # 2x MMs, same cols, same swaps
                            tspans = [sp for (j, c0, w, st) in spans for sp in
                                      ((j, c0, w // 2, st), (j, c0 + w // 2, w - w // 2, False))]
                        elif "swapmax" in probes:  # every MM a weight swap
                            tspans = [spans[i] for i in (0, 2, 4, 6, 1, 3, 5, 7)]
                            tspans = [(j, c0, w, i == 0) for i, (j, c0, w, _) in enumerate(tspans)]
                        nlast = len(tspans) - 1
                        for idx, (j, c0, w, st) in enumerate(tspans):
                            reps_mm = 2 if "wide" in probes else 1
                            for r in range(reps_mm):
                                nc.tensor.matmul(
                                    acc[:, ds(p0 + c0, w)],
                                    lhsT=xt[:, ds((0 if "samex" in probes else t * KS * TB + j * TB), TB)],
                                    rhs=wt[:, ds(j * F + c0, w)],
                                    start=st and r == 0,
                                    stop=(idx == nlast) and r == reps_mm - 1 and not has_bias,
                                )
                        if has_bias:
                            nc.tensor.matmul(
                                acc[:, ds(p0, F)],
                                lhsT=ones,
                                rhs=brow,
                                start=False,
                                stop=True,
                            )
                    o0 = tg * tgroup * FO
                    if "nopost" in probes:
                        # drain PSUM with one cheap op so the group is consumed
                        nc.vector.tensor_copy(ot[:, ds(o0, 50)], acc[:, ds(0, 50)])
                        continue
                    # 3D APs batch the post-ops across the group's banks
                    a3 = acc[:].rearrange("p (g x) -> p g x", g=tgroup)
                    ot3 = ot[:, ds(o0, tgroup * FO)].rearrange(
                        "p (g x) -> p g x", g=tgroup)
                    # [A D F] -> out cols [o1 o2 o3] (fp32 -> fp16)
                    nc.scalar.activation(
                        ot3[:, :, 0:256], a3[:, :, 0:256],
                        mybir.ActivationFunctionType.Copy,
                    )
                    # o1 = max(A,B,C), o2 = max(D,E): in-place maxes with the
                    # SBUF out tile as accumulator (one PSUM operand per op)
                    nc.vector.tensor_max(
                        ot3[:, :, 0:100], ot3[:, :, 0:100], a3[:, :, 256:356]
                    )
                    nc.vector.tensor_max(
                        ot3[:, :, 0:50], ot3[:, :, 0:50], a3[:, :, 356:406]
                    )
                # SBUF [p, (t f)] -> DRAM [p, t, f]: contiguous per partition.
                # Stores go on the ACT HWDGE ring: sharing the SP ring with
                # the x loads serializes load(s+1) behind store(s) (HWDGE is
                # FIFO per issuing engine).
                if "noout" not in probes:
                    getattr(nc, store_eng).dma_start(
                        o_d[s].rearrange("p t f -> p (t f)"), ot[:]
                    )
    nc.compile()
    return nc


def _pack_weights(W1, b1, W2, b2, W3, b3, variant="v10"):
    Wc = np.zeros((K, F), np.float32)
    W3f = W3.reshape(156, K)
    Wc[0:256, 0:50] = W1.T                    # A = y1h0
    Wc[0:256, 50:100] = W2[:, 0, :].T         # D = y2h0
    Wc[256:512, 50:100] = W2[:, 1, :].T
    Wc[:, 100:256] = W3f.T                    # F = o3
    Wc[256:512, 256:306] = W1.T               # B = y1h1
    Wc[256:512, 306:356] = W2[:, 0, :].T      # E = y2h1
    Wc[512:768, 306:356] = W2[:, 1, :].T
    Wc[512:768, 356:406] = W1.T               # C = y1h2
    bparts = [b1[:, 0], b2[:, 0], b3, b1[:, 1], b2[:, 1], b1[:, 2]]
    wnp = WDT[variant][1]
    wt = np.ascontiguousarray(
        Wc.reshape(KS, 128, F).transpose(1, 0, 2).reshape(128, KS * F)
    ).astype(wnp)
    brow = (
        np.concatenate(bparts + [np.ones(TB)])
        .astype(wnp)[None, :]
    )
    return wt, brow


def _unpack_o(o):
    """Device output [SUP, 128, TPS, FO] fp16 -> [B, FO] fp32 in batch order."""
    return (
        np.asarray(o).transpose(0, 2, 1, 3).reshape(B, FO).astype(np.float32)
    )


def _pack_x(x, variant="v10"):
    """FULL x [N,3,256,1] fp32 -> per-core [SUP, 128, TPS*KS*TB] arrays."""
    xs = np.asarray(x).reshape(N, K).astype(XDT[variant][1])
    out = []
    for c in range(NCORES):
        xc = xs[c * B : (c + 1) * B]
        # [s, t, b, j, p] -> [s, p, t, j, b]: channel-major 128-partition tiles
        arr = np.ascontiguousarray(
            xc.reshape(SUP, TPS, TB, KS, 128).transpose(0, 4, 1, 3, 2)
        ).reshape(SUP, 128, TPS * KS * TB)
        out.append(arr)
    return out


def kernel(x, W1, b1, W2, b2, W3, b3):
    variant = "v10"
    wt, brow = _pack_weights(
        np.asarray(W1, np.float32),
        np.asarray(b1, np.float32),
        np.asarray(W2, np.float32),
        np.asarray(b2, np.float32),
        np.asarray(W3, np.float32),
        np.asarray(b3, np.float32),
        variant=variant,
    )

    has_bias = bool(
        np.any(np.asarray(b1)) or np.any(np.asarray(b2)) or np.any(np.asarray(b3))
    )
    key = ("nc", variant, has_bias)
    if key not in _cache:
        _cache[key] = _build_nc(has_bias=has_bias, variant=variant, xbufs=3)
    nc = _cache[key]

    xcores = _pack_x(x, variant)
    in_maps = [{"x": xc, "w": wt, "b": brow} for xc in xcores]

    res = run_bass_kernel_spmd(nc, in_maps, list(range(NCORES)))

    out = np.concatenate([_unpack_o(res.results[c]["o"]) for c in range(NCORES)])
    return out[:, :, None, None]


# revision 14
# speedup vs baseline: 1.0073x; 1.0073x over previous
"""Trainium2 Bass kernel for nn_Conv2d_72052371357971.

Text-CNN style conv stack: three conv groups (k=1,2,3) over [N,3,256]
windows + per-group max-pool, concatenated to [N,256].

Strategy (pure data parallel across 8 NeuronCores):
  * All three conv groups fold into ONE [768, 406] weight matrix over the
    flattened window (3*256 channels), block-sparse by k-subtile support:
      A = y1h0 (j0,j1)   D = y2h0 (j0..j3)   F = o3 (j0..j5)
      B = y1h1 (j2,j3)   E = y2h1 (j2..j5)   C = y1h2 (j4,j5)
  * v10: ONE PSUM bank per batch tile, one accumulation chain.  PSUM
    `has_written` is per-element: the start=True matmul clears the bank;
    later accumulating matmuls OVERWRITE not-yet-written columns (first
    touch) and accumulate written ones.  Column order [A D F B E C] makes
    every k-subtile's active span contiguous with ZERO padding:
      j0/j1: cols   0:256  (A D F)
      j2/j3: cols  50:356  (D F B E)
      j4/j5: cols 100:256 + 306:406  (F, E C; two spans share one
             LDWEIGHTS since the stationary x-subtile is unchanged)
    1636 streamed columns/tile (vs 2436 dense) -- measured ~0.445 ns/col;
    weight swaps and LDWEIGHTS are fully hidden, issue cost ~4.5ns/MM.
  * x is fp8e3 (e3m4) -- halves HBM in-traffic (DMA would otherwise
    co-bottleneck: in+out share the ~350GB/s per-core HBM path).  W stays
    fp16; the PE accepts mixed fp8e3-stationary x fp16-moving operands at
    the same 1 col/cycle.  PSUM accumulates fp32.  Max rel err 1.0e-2
    (fp8e4 DoubleRow would halve PE time but measures 2.4e-2 > 2e-2).
  * Post-process per 128-row tile: ONE ScalarE 256-col copy PSUM->out
    ([A D F] -> [o1 o2 o3] slots), then VectorE max with [B E] (100 cols)
    and with [C] (50 cols) in place.  DMA streams out fp16 rows; host
    upcasts to fp32.
"""

import numpy as np

import concourse.bacc as bacc
import concourse.mybir as mybir
import concourse.tile as tile
from concourse.bass import ds
from concourse.bass_utils import run_bass_kernel_spmd

# Problem shapes (hardcoded per contract)
N = 65536
NCORES = 8
B = N // NCORES           # 8192 batch rows per core
TB = 128                  # batch tile (PSUM partition dim)
TPS = 8                   # batch tiles per super-tile
SUP = B // (TPS * TB)     # 8 super-tiles per core
K = 768                   # contraction: 3 positions x 256 channels
KS = K // 128             # 6 K-subtiles
F = 406                   # pre-pool filters: 3*50 + 2*50 + 156
FO = 256                  # output filters after pooling

import ml_dtypes

_F32 = mybir.dt.float32
_F16 = mybir.dt.float16
_F16_NP = np.float16
_BF16 = mybir.dt.bfloat16
_BF16_NP = ml_dtypes.bfloat16
_E3 = mybir.dt.float8e3
_E3_NP = ml_dtypes.float8_e3m4
_cache = {}

# v9 span schedule: (j, wcol0, ncols, start).  Weight-buffer column layout
# [A 0:50 | D 50:100 | F 100:256 | B 256:306 | E 306:356 | C 356:406],
# same order in every k-subtile segment.  Single PSUM bank; first-touch
# columns are overwritten via per-element has_written.
SPANS = {
    "v9": [
        (0, 0, 256, True),
        (1, 0, 256, False),
        (2, 50, 306, False),
        (3, 50, 306, False),
        (4, 100, 306, False),
        (5, 100, 306, False),
    ],
    # v10: j4/j5 split into two spans sharing one LDWEIGHTS (B-region pad
    # eliminated): 1636 streamed cols, 8 matmuls, 6 weight swaps.
    "v10": [
        (0, 0, 256, True),
        (1, 0, 256, False),
        (2, 50, 306, False),
        (3, 50, 306, False),
        (4, 100, 156, False), (4, 306, 100, False),
        (5, 100, 156, False), (5, 306, 100, False),
    ],
}
SPANS["v11"] = SPANS["v10"]
# x dtype per variant: fp8e3 (e3m4) halves HBM in-traffic; the PE matmul
# mixes fp8e3 stationary x with 16-bit moving weights at 1 col/cycle.
XDT = {"v9": (_F16, _F16_NP), "v10": (_E3, _E3_NP), "v11": (_E3, _E3_NP)}
WDT = {"v9": (_F16, _F16_NP), "v10": (_F16, _F16_NP), "v11": (_BF16, _BF16_NP)}


def _build_nc(
    reps=1,
    has_bias=True,
    variant="v9",
    xbufs=2,
    obufs=2,
    pbufs=4,
    tgroup=1,  # batch tiles per PSUM group (1: one bank/tile; 2: two banks,
               # post-ops batched across the pair via 3D strided APs)
    store_eng="scalar",  # engine issuing the output-store DMA
    probe=None,  # timing diagnostics, comma-separated: 'dma' no compute;
                 # 'noin' no x loads; 'noout' no out stores; 'nopost' no
                 # ACT/DVE post-ops; 'samex' all matmuls share one lhsT
):
    probes = set((probe or "").split(","))
    spans = SPANS[variant]
    xdt, _ = XDT[variant]
    wdt, _ = WDT[variant]
    nc = bacc.Bacc("TRN2", target_bir_lowering=False, debug=False)

    x_d = nc.dram_tensor("x", [SUP, 128, TPS * KS * TB], xdt, kind="ExternalInput")
    w_d = nc.dram_tensor("w", [128, KS * F], wdt, kind="ExternalInput")
    # bias row and a ones row for the K=1 bias matmul
    b_d = nc.dram_tensor("b", [1, F + TB], wdt, kind="ExternalInput")
    # p-major layout: the store is contiguous per partition (4KB chunks);
    # host reorders [s,p,t,f] -> batch order for free
    o_d = nc.dram_tensor("o", [SUP, 128, TPS, FO], _F16, kind="ExternalOutput")

    with tile.TileContext(nc) as tc:
        with (
            tc.tile_pool(name="const", bufs=1) as constp,
            tc.tile_pool(name="xp", bufs=xbufs) as xp,
            tc.tile_pool(name="op", bufs=obufs) as op,
            tc.tile_pool(name="ps", bufs=pbufs, space="PSUM") as psp,
        ):
            wt = constp.tile([128, KS * F], wdt)
            nc.sync.dma_start(wt[:], w_d[:])
            if "noin" in probes:
                # single resident x tile: marginal rep traffic excludes loads
                xt0 = constp.tile([128, TPS * KS * TB], xdt)
                nc.sync.dma_start(xt0[:], x_d[0])
            if has_bias:
                bt = constp.tile([1, F + TB], wdt)
                nc.sync.dma_start(bt[:], b_d[:])
                brow = bt[:, ds(0, F)]
                ones = bt[:, ds(F, TB)]

            for s in [si for _ in range(reps) for si in range(SUP)]:
                if "noin" in probes:
                    xt = xt0
                else:
                    xt = xp.tile([128, TPS * KS * TB], xdt)
                    nc.sync.dma_start(xt[:], x_d[s])
                ot = op.tile([128, TPS * FO], _F16)
                if "dma" in probes:
                    # trivial write so the out store has a defined producer
                    nc.vector.tensor_copy(ot[:], xt[:, ds(0, TPS * FO)])
                for tg in range(TPS // tgroup) if "dma" not in probes else []:
                    acc = psp.tile([128, 512 * tgroup], _F32, tag="ps", name="acc")
                    for ti in range(tgroup):
                        t = tg * tgroup + ti
                        p0 = ti * 512
                        tspans = spans
                        if "split" in probes:  # 2x MMs, same cols, same swaps
                            tspans = [sp for (j, c0, w, st) in spans for sp in
                                      ((j, c0, w // 2, st), (j, c0 + w // 2, w - w // 2, False))]
                        elif "swapmax" in probes:  # every MM a weight swap
                            tspans = [spans[i] for i in (0, 2, 4, 6, 1, 3, 5, 7)]
                            tspans = [(j, c0, w, i == 0) for i, (j, c0, w, _) in enumerate(tspans)]
                        nlast = len(tspans) - 1
                        for idx, (j, c0, w, st) in enumerate(tspans):
                            reps_mm = 2 if "wide" in probes else 1
                            for r in range(reps_mm):
                                nc.tensor.matmul(
                                    acc[:, ds(p0 + c0, w)],
                                    lhsT=xt[:, ds((0 if "samex" in probes else t * KS * TB + j * TB), TB)],
                                    rhs=wt[:, ds(j * F + c0, w)],
                                    start=st and r == 0,
                                    stop=(idx == nlast) and r == reps_mm - 1 and not has_bias,
                                )
                        if has_bias:
                            nc.tensor.matmul(
                                acc[:, ds(p0, F)],
                                lhsT=ones,
                                rhs=brow,
                                start=False,
                                stop=True,
                            )
                    o0 = tg * tgroup * FO
                    if "nopost" in probes:
                        # drain PSUM with one cheap op so the group is consumed
                        nc.vector.tensor_copy(ot[:, ds(o0, 50)], acc[:, ds(0, 50)])
                        continue
                    # 3D APs batch the post-ops across the group's banks
                    a3 = acc[:].rearrange("p (g x) -> p g x", g=tgroup)
                    ot3 = ot[:, ds(o0, tgroup * FO)].rearrange(
                        "p (g x) -> p g x", g=tgroup)
                    # [A D F] -> out cols [o1 o2 o3] (fp32 -> fp16)
                    nc.scalar.activation(
                        ot3[:, :, 0:256], a3[:, :, 0:256],
                        mybir.ActivationFunctionType.Copy,
                    )
                    # o1 = max(A,B,C), o2 = max(D,E): in-place maxes with the
                    # SBUF out tile as accumulator (one PSUM operand per op)
                    nc.vector.tensor_max(
                        ot3[:, :, 0:100], ot3[:, :, 0:100], a3[:, :, 256:356]
                    )
                    nc.vector.tensor_max(
                        ot3[:, :, 0:50], ot3[:, :, 0:50], a3[:, :, 356:406]
                    )
                # SBUF [p, (t f)] -> DRAM [p, t, f]: contiguous per partition.
                # Stores go on the ACT HWDGE ring: sharing the SP ring with
                # the x loads serializes load(s+1) behind store(s) (HWDGE is
                # FIFO per issuing engine).
                if "noout" not in probes:
                    getattr(nc, store_eng).dma_start(
                        o_d[s].rearrange("p t f -> p (t f)"), ot[:]
                    )
    nc.compile()
    return nc


def _pack_weights(W1, b1, W2, b2, W3, b3, variant="v10"):
    Wc = np.zeros((K, F), np.float32)
    W3f = W3.reshape(156, K)
    Wc[0:256, 0:50] = W1.T                    # A = y1h0
    Wc[0:256, 50:100] = W2[:, 0, :].T         # D = y2h0
    Wc[256:512, 50:100] = W2[:, 1, :].T
    Wc[:, 100:256] = W3f.T                    # F = o3
    Wc[256:512, 256:306] = W1.T               # B = y1h1
    Wc[256:512, 306:356] = W2[:, 0, :].T      # E = y2h1
    Wc[512:768, 306:356] = W2[:, 1, :].T
    Wc[512:768, 356:406] = W1.T               # C = y1h2
    bparts = [b1[:, 0], b2[:, 0], b3, b1[:, 1], b2[:, 1], b1[:, 2]]
    wnp = WDT[variant][1]
    wt = np.ascontiguousarray(
        Wc.reshape(KS, 128, F).transpose(1, 0, 2).reshape(128, KS * F)
    ).astype(wnp)
    brow = (
        np.concatenate(bparts + [np.ones(TB)])
        .astype(wnp)[None, :]
    )
    return wt, brow


def _unpack_o(o):
    """Device output [SUP, 128, TPS, FO] fp16 -> [B, FO] fp32 in batch order."""
    return (
        np.asarray(o).transpose(0, 2, 1, 3).reshape(B, FO).astype(np.float32)
    )


def _pack_x(x, variant="v10"):
    """FULL x [N,3,256,1] fp32 -> per-core [SUP, 128, TPS*KS*TB] arrays."""
    xs = np.asarray(x).reshape(N, K).astype(XDT[variant][1])
    out = []
    for c in range(NCORES):
        xc = xs[c * B : (c + 1) * B]
        # [s, t, b, j, p] -> [s, p, t, j, b]: channel-major 128-partition tiles
        arr = np.ascontiguousarray(
            xc.reshape(SUP, TPS, TB, KS, 128).transpose(0, 4, 1, 3, 2)
        ).reshape(SUP, 128, TPS * KS * TB)
        out.append(arr)
    return out


def kernel(x, W1, b1, W2, b2, W3, b3):
    variant = "v10"
    wt, brow = _pack_weights(
        np.asarray(W1, np.float32),
        np.asarray(b1, np.float32),
        np.asarray(W2, np.float32),
        np.asarray(b2, np.float32),
        np.asarray(W3, np.float32),
        np.asarray(b3, np.float32),
        variant=variant,
    )

    has_bias = bool(
        np.any(np.asarray(b1)) or np.any(np.asarray(b2)) or np.any(np.asarray(b3))
    )
    key = ("nc", variant, has_bias)
    if key not in _cache:
        _cache[key] = _build_nc(has_bias=has_bias, variant=variant, xbufs=3, pbufs=8)
    nc = _cache[key]

    xcores = _pack_x(x, variant)
    in_maps = [{"x": xc, "w": wt, "b": brow} for xc in xcores]

    res = run_bass_kernel_spmd(nc, in_maps, list(range(NCORES)))

    out = np.concatenate([_unpack_o(res.results[c]["o"]) for c in range(NCORES)])
    return out[:, :, None, None]
